# revision 1
# baseline (speedup 1.0000x reference)
"""
Multi-head attention (b=4, n=1024, e=768, h=12, dh=64) on 8 trn2 NeuronCores.

Sharding: (batch, head-group) -> core.  Core c handles batch b=c//2 and head
group g=c%2 (6 of the 12 heads).  Each core computes QKV projection for its
heads, attention, and a row-parallel slice of the output projection, producing
a partial [1024, 768] output.  The host sums the two partials per batch
(the row-parallel all-reduce) during unsharding.

All biases ride an augmented contraction row: inputs are padded to 896 (=7*128)
contraction rows where row 768 of xT is ones and row 768 of each weight holds
the bias (rows 769+ are zero).  The softmax denominator comes from an extra
ones-column appended to each head's V (column 64 of each 65-wide head slot),
so the column-sum of exp(E) falls out of the same PE matmul that computes A@V.
The softmax 1/sqrt(768) scale is folded into Wproj on the host.

Matmuls run as float32r (TF32 fast path on the PE).  The E matmuls are
zero-padded to a full 128-row contraction: half-array matmuls (K=64) keep the
PE's HAM activity monitor below its warm threshold and the attention phase
gets clocked at 1.2 GHz instead of 2.4.  Junk warmup/bridge matmuls keep the
clock-gate warm across the initial DMA wait and phase transitions.
"""

import math
from contextlib import ExitStack

import numpy as np

import concourse.mybir as mybir
import concourse.tile as tile
from concourse import bacc
from concourse.bass_utils import run_bass_kernel_spmd

EMB = 768
HEADS = 12
DH = 64
N = 1024
B = 4
HPC = 6  # heads per core
KC = 7  # contraction chunks (896 = 7*128) for the bias-augmented qkv matmuls
VW = HPC * 65 + 63  # V slots overlap: lhsT reads 128 cols from slot h*65
F32 = mybir.dt.float32
F32R = mybir.dt.float32r

N_CORES = 8


def build_program():
    nc = bacc.Bacc("TRN2", target_bir_lowering=False, debug=False, num_devices=N_CORES)

    xT = nc.dram_tensor("xT", [KC * 128, N], F32R, kind="ExternalInput").ap()
    Wq = nc.dram_tensor("Wq", [KC * 128, 384], F32R, kind="ExternalInput").ap()
    Wk = nc.dram_tensor("Wk", [KC * 128, 384], F32R, kind="ExternalInput").ap()
    Wv = nc.dram_tensor("Wv", [KC * 128, HPC * 65], F32R, kind="ExternalInput").ap()
    Wp = nc.dram_tensor("Wp", [4 * 128, EMB], F32R, kind="ExternalInput").ap()
    out = nc.dram_tensor("out", [N, EMB], F32, kind="ExternalOutput").ap()

    Exp = mybir.ActivationFunctionType.Exp

    with tile.TileContext(nc) as tc, ExitStack() as ctx:
        const = ctx.enter_context(tc.tile_pool(name="const", bufs=1))
        ldp = tc.alloc_tile_pool(name="ldp", bufs=1)

        # Phase-1-only SBUF tensors (pool released after the QKV projections)
        xT_sb = ldp.tile([128, KC, N], F32R)
        Wq_sb = ldp.tile([128, KC, 384], F32R)
        Wk_sb = ldp.tile([128, KC, 384], F32R)
        Wv_sb = ldp.tile([128, KC, HPC * 65], F32R)
        # Resident
        Wp_sb = const.tile([128, 4, EMB], F32R)
        # warmup operand, built by the first two DVE ops so the PE can start
        # its HAM-warmup junk matmuls ~3us in (a DMA-fed warm tile only lands
        # at ~10us: queue bring-up dominates)
        warm_f32 = const.tile([128, 640], F32)
        nc.vector.memset(warm_f32[:], 0.125)
        warm_sb = const.tile([128, 640], F32R)
        nc.vector.tensor_copy(warm_sb[:], warm_f32[:])
        # per-chunk DMAs so QKV matmuls can start before the full load lands
        xT_r = xT.rearrange("(c p) n -> p c n", p=128)
        Wq_r = Wq.rearrange("(c p) n -> p c n", p=128)
        Wk_r = Wk.rearrange("(c p) n -> p c n", p=128)
        Wv_r = Wv.rearrange("(c p) n -> p c n", p=128)
        for c in range(KC):
            nc.sync.dma_start(xT_sb[:, c, :], xT_r[:, c, :])
            nc.sync.dma_start(Wq_sb[:, c, :], Wq_r[:, c, :])
            nc.sync.dma_start(Wk_sb[:, c, :], Wk_r[:, c, :])
            if c >= 3:
                nc.sync.dma_start(Wv_sb[:, c - 3, :], Wv_r[:, c - 3, :])
        for c in range(KC - 3, KC):
            nc.sync.dma_start(Wv_sb[:, c, :], Wv_r[:, c, :])
        nc.sync.dma_start(Wp_sb[:], Wp.rearrange("(c p) n -> p c n", p=128))

        # Per-head padded Q^T/K^T: head h in partitions 0..63, zeros in 64..127
        # so attention matmuls present full 128x128 stationary shapes to the PE
        # (half-array shapes keep the HAM throttle engaged).
        QT_sb = const.tile([128, HPC, N], F32R)
        KT_sb = const.tile([128, HPC, N], F32R)
        V_sb = const.tile([128, 8, VW], F32R)  # V natural, 8 token chunks
        OT_sb = const.tile([128, 4, N], F32R)  # scaled O^T; chunk 3 = [ones; zeros]

        # Zero the padded partitions of QT/KT (both sides must be real zeros:
        # 0 * NaN-garbage would still poison the contraction).  Rows 769.. of
        # xT are zeros by construction, so DMA them in instead of burning DVE
        # time.  The V slot padding columns only feed PSUM rows 65..127, which
        # are never read, so they can stay uninitialized.
        ones_scratch = const.tile([128, N], F32)
        nc.vector.memset(ones_scratch[:], 0.0)
        nc.vector.memset(ones_scratch[0:1, :], 1.0)
        nc.vector.tensor_copy(OT_sb[:, 3, :], ones_scratch[:])
        for h in range(HPC):
            nc.vector.tensor_copy(QT_sb[64:128, h, :], ones_scratch[64:128, :])
            nc.vector.tensor_copy(KT_sb[64:128, h, :], ones_scratch[64:128, :])
        for t8 in range(8):
            nc.vector.tensor_copy(V_sb[:, t8, HPC * 65 :], ones_scratch[:, 0:63])

        # ---- Phase 1: QKV projections -------------------------------------
        with tc.tile_pool(name="psum1", bufs=1, space="PSUM") as pp1:
            # warmup junk matmuls: the PE would otherwise idle for ~10us while
            # the input DMAs land, leaving the HAM clock-gate cold for the
            # first half of the QKV phase
            for w in range(8):
                pw = pp1.tile([128, 512], F32, tag="qk", bufs=6, name=f"pw_{w}")
                nc.tensor.matmul(
                    pw[:], warm_sb[:, 0:128], warm_sb[:, 128:640],
                    start=True, stop=True,
                )
            def emit_v_group(t8):
                psv = pp1.tile([128, HPC * 65], F32, tag="v", bufs=2, name=f"psv_{t8}")
                for c in range(KC):
                    nc.tensor.matmul(
                        psv[:],
                        xT_sb[:, c, t8 * 128 : (t8 + 1) * 128],
                        Wv_sb[:, c, :],
                        start=(c == 0),
                        stop=(c == KC - 1),
                    )
                nc.vector.tensor_copy(V_sb[:, t8, 0 : HPC * 65], psv[:])

            groups = [(p3, qc) for p3 in range(3) for qc in range(2)]
            for W_sb, T_sb in ((Wq_sb, QT_sb), (Wk_sb, KT_sb)):
                gt = [
                    pp1.tile([128, 512], F32, tag="qk", bufs=6, name=f"g_{i}")
                    for i in range(6)
                ]
                for c in range(KC):
                    for i, (p3, qc) in enumerate(groups):
                        nc.tensor.matmul(
                            gt[i],
                            W_sb[:, c, p3 * 128 : (p3 + 1) * 128],
                            xT_sb[:, c, qc * 512 : (qc + 1) * 512],
                            start=(c == 0),
                            stop=(c == KC - 1),
                        )
                for i, (p3, qc) in enumerate(groups):
                    nc.vector.tensor_copy(
                        T_sb[0:64, 2 * p3, qc * 512 : (qc + 1) * 512],
                        gt[i][0:64, :],
                    )
                    nc.scalar.copy(
                        T_sb[0:64, 2 * p3 + 1, qc * 512 : (qc + 1) * 512],
                        gt[i][64:128, :],
                    )
            if True:
                for t8 in range(8):
                    emit_v_group(t8)
        ldp.release()

        # ---- Phase 2: attention per head ----------------------------------
        expp = ctx.enter_context(tc.tile_pool(name="expp", bufs=4))
        rpool = ctx.enter_context(tc.tile_pool(name="rpool", bufs=2))
        oupool = ctx.enter_context(tc.tile_pool(name="oupool", bufs=3))
        with tc.tile_pool(name="psum2", bufs=1, space="PSUM") as pp2:

            def emit_e_chunk(h, kc, junk=False):
                """E^T block (k-chunk kc) for head h -> exp tile in SBUF.

                junk=True emits the matmuls but no exp and returns None: used
                at the end of the attention phase purely to keep the PE's HAM
                activity monitor warm through the proj-phase transition."""
                pe = pp2.tile([128, N], F32, tag="e", bufs=3, name=f"pe_{h}_{kc}")
                for qc in range(2):
                    nc.tensor.matmul(
                        pe[:, qc * 512 : (qc + 1) * 512],
                        KT_sb[:, h, kc * 128 : (kc + 1) * 128],
                        QT_sb[:, h, qc * 512 : (qc + 1) * 512],
                        start=True,
                        stop=True,
                    )
                if junk:
                    return None
                ex = expp.tile([128, N], F32R, tag="ex", name=f"ex_{h}_{kc}")
                nc.scalar.activation(ex[:], pe[:], Exp)
                return ex

            # software pipeline: E-matmuls run one chunk ahead of AV-matmuls so
            # the PE always has exp-independent work while ACT computes exp.
            ex_next = emit_e_chunk(0, 0)
            for h in range(HPC):
                p3, half = divmod(h, 2)
                off = 64 * half
                po = pp2.tile([128, N], F32, tag="o", bufs=1, name=f"po_{h}")
                for kc in range(8):
                    ex = ex_next
                    nh, nkc = (h, kc + 1) if kc < 7 else (h + 1, 0)
                    if nh < HPC:
                        ex_next = emit_e_chunk(nh, nkc)
                    else:
                        # keepalive: dense PE work through the pipeline drain
                        emit_e_chunk(HPC - 1, nkc if kc < 7 else 0, junk=True)
                    for qc in range(2):
                        nc.tensor.matmul(
                            po[0:65, qc * 512 : (qc + 1) * 512],
                            V_sb[:, kc, h * 65 : h * 65 + 65],
                            ex[:, qc * 512 : (qc + 1) * 512],
                            start=(kc == 0),
                            stop=(kc == 7),
                        )
                # softmax normalization: row 64 of po holds sum_k exp(E^T).
                # Copy the raw block out of PSUM first so the po banks free up
                # for the next head's AV accumulation, then normalize from
                # SBUF off the critical path.  approx-fast reciprocal (~4e-6
                # rel err) is far below the TF32 matmul noise; it misreads
                # nonzero partition offsets on HW, so the sums row gets its
                # own partition-0 copy.
                ou = oupool.tile([65, N], F32, tag="ou", name=f"ou_{h}")
                nc.vector.tensor_copy(ou[:], po[0:65, :])
                ss = rpool.tile([1, N], F32, tag="ss")
                nc.vector.tensor_copy(ss[:], ou[64:65, :])
                rs = rpool.tile([1, N], F32, tag="rs")
                nc.vector.reciprocal_approx_fast(rs[:], ss[:])
                rb = rpool.tile([64, N], F32, tag="rb")
                nc.gpsimd.partition_broadcast(rb[:], rs[:])
                nc.vector.tensor_mul(
                    OT_sb[off : off + 64, p3, :], ou[0:64, :], rb[:]
                )
            # bridge the last head's normalization chain (PE would idle ~4us
            # before the proj phase's head-4/5 matmuls, re-engaging the
            # throttle; the c-order (3,0,1,2) proj groups cover part of it)
            for w in range(4):
                emit_e_chunk(HPC - 1, w % 8, junk=True)

        # ---- Phase 3: output projection -----------------------------------
        outp = ctx.enter_context(tc.tile_pool(name="outp", bufs=3))
        with tc.tile_pool(name="psum3", bufs=1, space="PSUM") as pp3:
            for qc8 in range(8):
                pso = pp3.tile([128, EMB], F32, tag="p", bufs=3)
                for n0, n1 in ((0, 512), (512, 768)):
                    for ci, c in enumerate((3, 0, 1, 2)):
                        nc.tensor.matmul(
                            pso[:, n0:n1],
                            OT_sb[:, c, qc8 * 128 : (qc8 + 1) * 128],
                            Wp_sb[:, c, n0:n1],
                            start=(ci == 0),
                            stop=(ci == 3),
                        )
                ot = outp.tile([128, EMB], F32, tag="out")
                nc.vector.tensor_copy(ot[:, 0:384], pso[:, 0:384])
                nc.scalar.copy(ot[:, 384:768], pso[:, 384:768])
                nc.sync.dma_start(out[qc8 * 128 : (qc8 + 1) * 128, :], ot[:])

    nc.compile()
    return nc


def build_in_maps(x, Wqkv, bqkv, Wproj, bproj):
    x = np.asarray(x, dtype=np.float32)
    Wqkv = np.asarray(Wqkv, dtype=np.float32)
    bqkv = np.asarray(bqkv, dtype=np.float32)
    Wproj = np.asarray(Wproj, dtype=np.float32)
    bproj = np.asarray(bproj, dtype=np.float32)

    s = 1.0 / math.sqrt(EMB)
    cols = np.arange(3 * EMB).reshape(HEADS, DH, 3)  # (h, d, qkv) col index map
    in_maps = []
    for c in range(N_CORES):
        b, g = divmod(c, 2)
        hsl = slice(g * HPC, (g + 1) * HPC)
        qcols = cols[hsl, :, 0].reshape(-1)
        kcols = cols[hsl, :, 1].reshape(-1)
        vcols = cols[hsl, :, 2]  # [HPC, DH]

        xT_a = np.zeros((KC * 128, N), np.float32)
        xT_a[:EMB] = x[b].T
        xT_a[EMB] = 1.0

        Wq_a = np.zeros((KC * 128, 384), np.float32)
        Wq_a[:EMB] = Wqkv[:, qcols]
        Wq_a[EMB] = bqkv[qcols]
        Wk_a = np.zeros((KC * 128, 384), np.float32)
        Wk_a[:EMB] = Wqkv[:, kcols]
        Wk_a[EMB] = bqkv[kcols]

        Wv_a = np.zeros((KC * 128, HPC * 65), np.float32)
        for j in range(HPC):
            Wv_a[:EMB, j * 65 : j * 65 + DH] = Wqkv[:, vcols[j]]
            Wv_a[EMB, j * 65 : j * 65 + DH] = bqkv[vcols[j]]
            Wv_a[EMB, j * 65 + DH] = 1.0

        Wp_a = np.zeros((4 * 128, EMB), np.float32)
        Wp_a[:384] = Wproj[g * 384 : (g + 1) * 384] * s
        if g == 0:
            Wp_a[384] = bproj

        in_maps.append({"xT": xT_a, "Wq": Wq_a, "Wk": Wk_a, "Wv": Wv_a, "Wp": Wp_a})
    return in_maps


_NC_CACHE = None


def _get_program():
    global _NC_CACHE
    if _NC_CACHE is None:
        _NC_CACHE = build_program()
    return _NC_CACHE


def kernel(x, Wqkv, bqkv, Wproj, bproj, **_kwargs):
    nc = _get_program()
    in_maps = build_in_maps(x, Wqkv, bqkv, Wproj, bproj)
    res = run_bass_kernel_spmd(nc, in_maps, list(range(N_CORES))).results
    out = np.empty((B, N, EMB), np.float32)
    for b in range(B):
        out[b] = res[2 * b]["out"] + res[2 * b + 1]["out"]
    return out



# revision 12
# speedup vs baseline: 1.1578x; 1.1578x over previous
"""
Multi-head attention (b=4, n=1024, e=768, h=12, dh=64) on 8 trn2 NeuronCores.

Sharding: (batch, head-group) -> core.  Core c handles batch b=c//2 and head
group g=c%2 (6 of the 12 heads).  Each core computes QKV projection for its
heads, attention, and a row-parallel slice of the output projection, producing
a partial [1024, 768] output.  The host sums the two partials per batch
(the row-parallel all-reduce) during unsharding.

v2 layout/schedule (vs the v1 fp32r kernel):
- fp16 inputs.  x^T, Wq, Wk ride ONE packed dram tensor (6 chunk-major DMAs
  instead of 21) to cut both bytes and the ~620ns/issue sync-engine
  serialization.  Wv, Wp are single fp16 DMAs.
- 6 contraction chunks (768 rows, no bias row).  Q/K biases are added by DVE
  during the PSUM->SBUF copies (per-partition tensor_scalar); the V and proj
  biases are folded into a host-side constant row (exact: softmax rows sum
  to 1) added during unsharding.
- exp() output and V are bf16 (E reaches ~75, fp16 would overflow);
  Q/K/x/W are fp16 (same 10-bit mantissa the old fp32r path kept).
- The gpsimd library and the ACT exp table are prewarmed at kernel start
  (v1 lazy-loaded both mid-kernel, costing a ~6us pipeline stall).
- QKV-phase PE work is interleaved INTO the attention phase: E(head0) runs
  right after the p3=0 Q/K blocks finish, so the ACT engine (the 55us exp
  bottleneck) starts ~35us earlier than v1's phase-sequential schedule.
- Softmax denominators come from a ones-column in each V slot.  AV PSUM is
  split into qc-halves so the next head's AV can start as soon as the first
  half is drained.  Head 5's reciprocal broadcast runs on the PE (ones
  matmul) so the output projection starts right behind it.
"""

import math
from contextlib import ExitStack

import numpy as np

import concourse.mybir as mybir
import concourse.tile as tile
from concourse import bacc
from concourse.bass_utils import run_bass_kernel_spmd

EMB = 768
HEADS = 12
DH = 64
N = 1024
B = 4
HPC = 6  # heads per core
CC = 6  # contraction chunks (768 = 6*128)
XO = 0  # xT cols in IN
QO = 1024  # Wq cols
KO = 1408  # Wk cols
BO = 1792  # bias cols (chunk 0 rows): q-bias head 0..5 | pad2 | k-bias head 0..5
INW = BO + 16
VW = HPC * 65 + 63  # V slots overlap: lhsT reads 128 cols from slot h*65
F32 = mybir.dt.float32
F32R = mybir.dt.float32r
F16 = mybir.dt.float16
BF16 = mybir.dt.bfloat16

N_CORES = 8
DEBUG = False


def build_program():
    nc = bacc.Bacc("TRN2", target_bir_lowering=False, debug=False, num_devices=N_CORES)

    IN = nc.dram_tensor("IN", [CC * 128, INW], F16, kind="ExternalInput").ap()
    Wv = nc.dram_tensor("Wv", [CC * 128, HPC * 65], F16, kind="ExternalInput").ap()
    Wp = nc.dram_tensor("Wp", [3 * 128, EMB], F16, kind="ExternalInput").ap()
    out = nc.dram_tensor("out", [N, EMB], F32, kind="ExternalOutput").ap()
    if DEBUG:
        dQT = nc.dram_tensor("dQT", [HPC * 128, N], F32, kind="ExternalOutput").ap()
        dKT = nc.dram_tensor("dKT", [HPC * 128, N], F32, kind="ExternalOutput").ap()
        dV = nc.dram_tensor("dV", [8 * 128, VW], F32, kind="ExternalOutput").ap()
        dOT = nc.dram_tensor("dOT", [3 * 128, N], F32, kind="ExternalOutput").ap()
        dEX = nc.dram_tensor("dEX", [128, N], F32, kind="ExternalOutput").ap()
        dOU = nc.dram_tensor("dOU", [128, N], F32, kind="ExternalOutput").ap()
        dPE = nc.dram_tensor("dPE", [128, N], F32, kind="ExternalOutput").ap()
        dRB = nc.dram_tensor("dRB", [128, N], F32, kind="ExternalOutput").ap()

    Exp = mybir.ActivationFunctionType.Exp

    with tile.TileContext(nc) as tc, ExitStack() as ctx:
        const = ctx.enter_context(tc.tile_pool(name="const", bufs=1))

        # ---- resident SBUF ------------------------------------------------
        IN_sb = const.tile([128, CC, INW], F16)
        Wv_sb = const.tile([128, CC, HPC * 65], F16)
        Wp_sb = const.tile([128, 3, EMB], F16)
        QT_sb = const.tile([128, HPC, N], F16)  # head h in parts 0..63, pad 64..127
        KT_sb = const.tile([128, HPC, N], F16)
        V_sb = const.tile([128, 8, VW], BF16)  # V natural, 8 token chunks
        OT_sb = const.tile([128, 3, N], F16)  # normalized O^T, head pair per chunk
        bias_sb = const.tile([128, 16], F32)
        warm = const.tile([128, 640], F16)  # junk matmul operand
        dummy = const.tile([128, 8], F32)  # prewarm sources
        dummy_a = const.tile([128, 8], F32)
        dummy_g = const.tile([128, 8], F32)
        ones64f = const.tile([1, 64], F32)
        ones64 = const.tile([1, 64], F32R)  # PE-broadcast stationary (head 5)
        if DEBUG:
            dbg_hold_ex = const.tile([128, N], F32)
            dbg_hold_pe = const.tile([128, N], F32)
            dbg_hold_ou = const.tile([128, N], F32)
            dbg_hold_rb = const.tile([128, N], F32)

        # prologue DVE work (engines otherwise idle during DMA bring-up)
        nc.vector.memset(dummy[:], 0.0)
        nc.vector.memset(warm[:], 0.125)
        nc.vector.memset(ones64f[:], 1.0)
        nc.vector.tensor_copy(ones64[:], ones64f[:])
        nc.vector.memset(QT_sb[64:128, :, :], 0.0)
        nc.vector.memset(KT_sb[64:128, :, :], 0.0)
        nc.vector.memset(V_sb[:, :, HPC * 65 :], 0.0)
        # ACT exp-table prewarm (table DMA ~2.7us) + gpsimd library prewarm
        nc.scalar.activation(dummy_a[0:1, :], dummy[0:1, :], Exp)
        nc.gpsimd.partition_broadcast(dummy_g[0:64, :], dummy[0:1, :])

        # ---- input DMAs (8 issues total) ----------------------------------
        IN_r = IN.rearrange("(c p) n -> p c n", p=128)
        for c in range(CC):
            nc.sync.dma_start(IN_sb[:, c, :], IN_r[:, c, :])
        nc.sync.dma_start(Wv_sb[:], Wv.rearrange("(c p) n -> p c n", p=128))
        nc.sync.dma_start(Wp_sb[:], Wp.rearrange("(c p) n -> p c n", p=128))
        # bias cast (waits on IN chunk 0)
        nc.vector.tensor_copy(bias_sb[:], IN_sb[:, 0, BO : BO + 16])

        # ---- PSUM pools (16KB/partition total) ----------------------------
        psO = ctx.enter_context(tc.tile_pool(name="psO", bufs=1, space="PSUM"))
        psA = tc.alloc_tile_pool(name="psA", bufs=1, space="PSUM")  # mm 2x2KB
        psB = tc.alloc_tile_pool(name="psB", bufs=1, space="PSUM")  # e 2x4KB

        expp = ctx.enter_context(tc.tile_pool(name="expp", bufs=10))
        oupool = ctx.enter_context(tc.tile_pool(name="oupool", bufs=3))
        rpool = ctx.enter_context(tc.tile_pool(name="rpool", bufs=2))

        # ---- emit helpers -------------------------------------------------
        def emit_junk(n, tag_tile):
            for _ in range(n):
                nc.tensor.matmul(
                    tag_tile[:, 0:512], warm[:, 0:128], warm[:, 128:640],
                    start=True, stop=True,
                )

        def qk_copy(g, qc, T_sb, p3, kbias):
            """PSUM group [128ch, 512 tok] -> two head slots (+ bias)."""
            bb = 8 if kbias else 0
            nc.vector.tensor_scalar_add(
                out=T_sb[0:64, 2 * p3, qc * 512 : (qc + 1) * 512],
                in0=g[0:64, :],
                scalar1=bias_sb[0:64, bb + 2 * p3 : bb + 2 * p3 + 1],
            )
            nc.vector.tensor_scalar_add(
                out=T_sb[0:64, 2 * p3 + 1, qc * 512 : (qc + 1) * 512],
                in0=g[64:128, :],
                scalar1=bias_sb[64:128, bb + 2 * p3 + 1 : bb + 2 * p3 + 2],
            )

        def emit_qk_group(p3, qc, wofs, T_sb, kbias, name):
            g = psA.tile([128, 512], F32, tag="mm", bufs=2, name=name)
            for c in range(CC):
                nc.tensor.matmul(
                    g[:],
                    IN_sb[:, c, wofs + p3 * 128 : wofs + (p3 + 1) * 128],
                    IN_sb[:, c, qc * 512 : (qc + 1) * 512],
                    start=(c == 0),
                    stop=(c == CC - 1),
                )
            qk_copy(g, qc, T_sb, p3, kbias)

        def emit_v_group(t8):
            g = psA.tile([128, 512], F32, tag="mm", bufs=2, name=f"psv_{t8}")
            for c in range(CC):
                nc.tensor.matmul(
                    g[:, 0 : HPC * 65],
                    IN_sb[:, c, t8 * 128 : (t8 + 1) * 128],
                    Wv_sb[:, c, :],
                    start=(c == 0),
                    stop=(c == CC - 1),
                )
            nc.vector.tensor_copy(V_sb[:, t8, 0 : HPC * 65], g[:, 0 : HPC * 65])
            # denominator ones-columns (the copy writes Wv's zero col there)
            nc.vector.memset(V_sb[:, t8, DH : HPC * 65 : 65], 1.0)

        def emit_e(h, kc):
            pe = psB.tile([128, N], F32, tag="e", bufs=2, name=f"pe_{h}_{kc}")
            for qc in range(2):
                nc.tensor.matmul(
                    pe[:, qc * 512 : (qc + 1) * 512],
                    KT_sb[:, h, kc * 128 : (kc + 1) * 128],
                    QT_sb[:, h, qc * 512 : (qc + 1) * 512],
                    start=True,
                    stop=True,
                )
            ex = expp.tile([128, N], BF16, tag="ex", name=f"ex_{h}_{kc}")
            nc.scalar.activation(ex[:], pe[:], Exp)
            if DEBUG and (h, kc) == (0, 0):
                nc.vector.tensor_copy(dbg_hold_ex[:], ex[:])
                nc.vector.tensor_copy(dbg_hold_pe[:], pe[:])
            return ex

        def begin_head(h):
            poa = psO.tile([65, 512], F32, tag="o2", bufs=2, name=f"poa_{h}")
            pob = psO.tile([65, 512], F32, tag="o2", bufs=2, name=f"pob_{h}")
            return (poa, pob)

        def emit_av(h, kc, po, ex):
            for qc in range(2):
                nc.tensor.matmul(
                    po[qc][0:65, :],
                    V_sb[:, kc, h * 65 : h * 65 + 65],
                    ex[:, qc * 512 : (qc + 1) * 512],
                    start=(kc == 0),
                    stop=(kc == 7),
                )

        def norm_chain(h, po, last=False):
            """po halves -> normalized OT chunk.  Ordered so the first PSUM
            half frees early (next head's AV reuses it) and rs lands early."""
            p3, half = divmod(h, 2)
            off = 64 * half
            ss = rpool.tile([1, N], F32, tag="ss", name=f"ss_{h}")
            ou = oupool.tile([65, N], F32, tag="ou", name=f"ou_{h}")
            rs = rpool.tile([1, N], F32, tag="rs", name=f"rs_{h}")
            if not last:
                nc.vector.tensor_copy(ss[:, 0:512], po[0][64:65, :])
                nc.vector.tensor_copy(ou[:, 0:512], po[0][0:65, :])
                nc.vector.tensor_copy(ou[:, 512:1024], po[1][0:65, :])
                nc.vector.tensor_copy(ss[:, 512:1024], po[1][64:65, :])
                if DEBUG and h == 0:
                    nc.vector.tensor_copy(dbg_hold_ou[0:65, :], ou[0:65, :])
                nc.vector.reciprocal_approx_fast(rs[:], ss[:])
                rb = rpool.tile([64, N], F32, tag="rb", name=f"rb_{h}")
                nc.gpsimd.partition_broadcast(rb[:], rs[:])
                nc.vector.tensor_mul(
                    OT_sb[off : off + 64, p3, :], ou[0:64, :], rb[:]
                )
                if DEBUG and h == 0:
                    nc.vector.tensor_copy(dbg_hold_rb[0:64, :], rb[:])
                return None
            # tail chain: sums first so rs (and the PE broadcast) land early
            nc.vector.tensor_copy(ss[:, 0:512], po[0][64:65, :])
            nc.vector.tensor_copy(ss[:, 512:1024], po[1][64:65, :])
            nc.vector.reciprocal_approx_fast(rs[:], ss[:])
            rsr = rpool.tile([1, N], F32R, tag="rsr", name="rsr")
            nc.vector.tensor_copy(rsr[:], rs[:])
            nc.vector.tensor_copy(ou[:, 0:512], po[0][0:65, :])
            nc.vector.tensor_copy(ou[:, 512:1024], po[1][0:65, :])
            return (ou, rsr, off, p3)

        # ---- phase 1a: junk + paced p3=0 Q groups -------------------------
        jt = psB.tile([128, N], F32, tag="e", bufs=2, name="junk")
        emit_junk(4, jt)
        gQ0 = psA.tile([128, 512], F32, tag="mm", bufs=2, name="gQ0")
        gQ1 = psA.tile([128, 512], F32, tag="mm", bufs=2, name="gQ1")
        for c in range(CC):
            nc.tensor.matmul(
                gQ0[:], IN_sb[:, c, QO : QO + 128], IN_sb[:, c, 0:512],
                start=(c == 0), stop=(c == CC - 1),
            )
            nc.tensor.matmul(
                gQ1[:], IN_sb[:, c, QO : QO + 128], IN_sb[:, c, 512:1024],
                start=(c == 0), stop=(c == CC - 1),
            )
            if c < CC - 1:
                emit_junk(1, jt)
        qk_copy(gQ0, 0, QT_sb, 0, False)
        qk_copy(gQ1, 1, QT_sb, 0, False)
        emit_qk_group(0, 0, KO, KT_sb, True, "gK0")
        emit_qk_group(0, 1, KO, KT_sb, True, "gK1")

        # ---- phase 1b/2: interleave remaining QKV with head-0/1 attention -
        # V groups first (AV head 0 needs them); p3=1,2 QK groups after
        # (E head 2+ needs them much later).
        fillers = [lambda t8=t8: emit_v_group(t8) for t8 in range(8)] + [
            lambda: emit_qk_group(1, 0, QO, QT_sb, False, "gQ2"),
            lambda: emit_qk_group(1, 1, QO, QT_sb, False, "gQ3"),
            lambda: emit_qk_group(1, 0, KO, KT_sb, True, "gK2"),
            lambda: emit_qk_group(1, 1, KO, KT_sb, True, "gK3"),
            lambda: emit_qk_group(2, 0, QO, QT_sb, False, "gQ4"),
            lambda: emit_qk_group(2, 1, QO, QT_sb, False, "gQ5"),
            lambda: emit_qk_group(2, 0, KO, KT_sb, True, "gK4"),
            lambda: emit_qk_group(2, 1, KO, KT_sb, True, "gK5"),
        ]
        fq = iter(fillers)

        def fill(n):
            for _ in range(n):
                f = next(fq, None)
                if f:
                    f()

        ex_tiles = {}
        ex_tiles[(0, 0)] = emit_e(0, 0)
        for kc in range(1, 8):
            fill(1)
            ex_tiles[(0, kc)] = emit_e(0, kc)
        fill(1)  # V7

        # head 0 AVs interleaved with head-1 E and remaining QK groups
        po = begin_head(0)
        for kc in range(8):
            ex_tiles[(1, kc)] = emit_e(1, kc)
            fill(1)
            emit_av(0, kc, po, ex_tiles.pop((0, kc)))
        fill(99)  # leftovers, if any
        norm_chain(0, po)

        # heads 1..4: E two chunks ahead so the first AV of each head starts
        # after the previous head's PSUM halves have both drained
        for h in range(1, 5):
            po = begin_head(h)
            ex_tiles[(h + 1, 0)] = emit_e(h + 1, 0)
            ex_tiles[(h + 1, 1)] = emit_e(h + 1, 1)
            for kc in range(8):
                if kc + 2 < 8:
                    ex_tiles[(h + 1, kc + 2)] = emit_e(h + 1, kc + 2)
                emit_av(h, kc, po, ex_tiles.pop((h, kc)))
            norm_chain(h, po)

        # head 5: straight AVs (exps already pipelined ahead)
        po = begin_head(5)
        for kc in range(8):
            emit_av(5, kc, po, ex_tiles.pop((5, kc)))
        tail = norm_chain(5, po, last=True)

        # release attention PSUM; proj pool takes the freed 12KB
        psB.release()
        psA.release()
        psD = ctx.enter_context(tc.tile_pool(name="psD", bufs=1, space="PSUM"))
        outp = ctx.enter_context(tc.tile_pool(name="outp", bufs=3))

        # ---- phase 3: output projection -----------------------------------
        def proj_c(pso, qc8, cs):
            for c in cs:
                for n0, n1 in ((0, 512), (512, 768)):
                    nc.tensor.matmul(
                        pso[:, n0:n1],
                        OT_sb[:, c, qc8 * 128 : (qc8 + 1) * 128],
                        Wp_sb[:, c, n0:n1],
                        start=(c == 0),
                        stop=(c == 2),
                    )

        def proj_finish(pso, qc8):
            ot = outp.tile([128, EMB], F32, tag="out", name=f"ot_{qc8}")
            nc.vector.tensor_copy(ot[:], pso[:])
            nc.sync.dma_start(out[qc8 * 128 : (qc8 + 1) * 128, :], ot[:])

        # head-5 chain tail overlap: c0/c1 passes for the first blocks run
        # while ss/rs/broadcast/mul complete; the broadcast is a PE ones
        # matmul (waits only on rs) so OT chunk 2 lands mid-pass.
        ou5, rsr5, off5, p35 = tail
        pso_t = {}
        for blk in range(2):
            pso_t[blk] = psD.tile([128, EMB], F32, tag="pso", bufs=3, name=f"pso_{blk}")
            proj_c(pso_t[blk], blk, (0, 1))
        rb5 = [
            psO.tile([64, 512], F32, tag="o2", bufs=2, name=f"rb5_{qc}")
            for qc in range(2)
        ]
        for qc in range(2):
            nc.tensor.matmul(
                rb5[qc][:],
                ones64[:],
                rsr5[:, qc * 512 : (qc + 1) * 512],
                start=True,
                stop=True,
            )
        for qc in range(2):
            nc.vector.tensor_mul(
                OT_sb[off5 : off5 + 64, p35, qc * 512 : (qc + 1) * 512],
                ou5[0:64, qc * 512 : (qc + 1) * 512],
                rb5[qc][:],
            )
        pso_t[2] = psD.tile([128, EMB], F32, tag="pso", bufs=3, name="pso_2")
        proj_c(pso_t[2], 2, (0, 1))
        for blk in range(3):
            proj_c(pso_t[blk], blk, (2,))
            proj_finish(pso_t[blk], blk)
        for blk in range(3, 8):
            pso = psD.tile([128, EMB], F32, tag="pso", bufs=3, name=f"pso_{blk}")
            proj_c(pso, blk, (0, 1, 2))
            proj_finish(pso, blk)

        if DEBUG:
            dbg = ctx.enter_context(tc.tile_pool(name="dbg", bufs=2))
            def dump(dst, src_ap, nparts, width):
                t = dbg.tile([128, width], F32, tag="d", bufs=2)
                nc.vector.tensor_copy(t[0:nparts, :], src_ap)
                nc.sync.dma_start(dst, t[0:nparts, :])
            for hh in range(HPC):
                dump(dQT[hh * 128 : (hh + 1) * 128, :], QT_sb[:, hh, :], 128, N)
                dump(dKT[hh * 128 : (hh + 1) * 128, :], KT_sb[:, hh, :], 128, N)
            for t8 in range(8):
                dump(dV[t8 * 128 : (t8 + 1) * 128, :], V_sb[:, t8, :], 128, VW)
            for cc in range(3):
                dump(dOT[cc * 128 : (cc + 1) * 128, :], OT_sb[:, cc, :], 128, N)
            dump(dEX, dbg_hold_ex[:], 128, N)
            dump(dOU, dbg_hold_ou[:], 128, N)
            dump(dPE, dbg_hold_pe[:], 128, N)
            dump(dRB, dbg_hold_rb[:], 128, N)

    nc.compile()
    return nc


def build_in_maps(x, Wqkv, bqkv, Wproj, bproj):
    x = np.asarray(x, dtype=np.float32)
    Wqkv = np.asarray(Wqkv, dtype=np.float32)
    bqkv = np.asarray(bqkv, dtype=np.float32)
    Wproj = np.asarray(Wproj, dtype=np.float32)

    s = 1.0 / math.sqrt(EMB)
    cols = np.arange(3 * EMB).reshape(HEADS, DH, 3)  # (h, d, qkv) col index map
    in_maps = []
    for core in range(N_CORES):
        b, g = divmod(core, 2)
        hsl = slice(g * HPC, (g + 1) * HPC)
        qcols = cols[hsl, :, 0].reshape(-1)
        kcols = cols[hsl, :, 1].reshape(-1)
        vcols = cols[hsl, :, 2]  # [HPC, DH]

        IN_a = np.zeros((CC * 128, INW), np.float16)
        IN_a[:, XO : XO + N] = x[b].T
        IN_a[:, QO : QO + 384] = Wqkv[:, qcols]
        IN_a[:, KO : KO + 384] = Wqkv[:, kcols]
        # per-head bias columns (chunk-0 rows = partitions), duplicated in
        # both partition halves so either scalar-operand alignment works
        bq = bqkv[qcols].reshape(HPC, DH)
        bk = bqkv[kcols].reshape(HPC, DH)
        for j in range(HPC):
            IN_a[0:64, BO + j] = bq[j]
            IN_a[64:128, BO + j] = bq[j]
            IN_a[0:64, BO + 8 + j] = bk[j]
            IN_a[64:128, BO + 8 + j] = bk[j]

        Wv_a = np.zeros((CC * 128, HPC * 65), np.float16)
        for j in range(HPC):
            Wv_a[:, j * 65 : j * 65 + DH] = Wqkv[:, vcols[j]]

        Wp_a = (Wproj[g * 384 : (g + 1) * 384] * s).astype(np.float16)

        in_maps.append({"IN": IN_a, "Wv": Wv_a, "Wp": Wp_a})
    return in_maps


_NC_CACHE = None


def _get_program():
    global _NC_CACHE
    if _NC_CACHE is None:
        _NC_CACHE = build_program()
    return _NC_CACHE


def kernel(x, Wqkv, bqkv, Wproj, bproj, **_kwargs):
    nc = _get_program()
    in_maps = build_in_maps(x, Wqkv, bqkv, Wproj, bproj)
    res = run_bass_kernel_spmd(nc, in_maps, list(range(N_CORES))).results
    # V-bias and proj-bias fold (exact: normalized softmax rows sum to 1)
    bqkv64 = np.asarray(bqkv, dtype=np.float64)
    vcols = (np.arange(3 * EMB).reshape(HEADS, DH, 3))[:, :, 2].reshape(-1)
    vec = (
        bqkv64[vcols] @ np.asarray(Wproj, dtype=np.float64) / math.sqrt(EMB)
        + np.asarray(bproj, dtype=np.float64)
    ).astype(np.float32)
    out = np.empty((B, N, EMB), np.float32)
    for b in range(B):
        out[b] = res[2 * b]["out"] + res[2 * b + 1]["out"] + vec
    return out


# revision 14
# speedup vs baseline: 1.2046x; 1.0404x over previous
"""
Multi-head attention (b=4, n=1024, e=768, h=12, dh=64) on 8 trn2 NeuronCores.

Sharding: (batch, head-group) -> core.  Core c handles batch b=c//2 and head
group g=c%2 (6 of the 12 heads).  Each core computes QKV projection for its
heads, attention, and a row-parallel slice of the output projection, producing
a partial [1024, 768] output.  The host sums the two partials per batch
(the row-parallel all-reduce) during unsharding.

v2 layout/schedule (vs the v1 fp32r kernel):
- fp16 inputs.  x^T, Wq, Wk ride ONE packed dram tensor (6 chunk-major DMAs
  instead of 21) to cut both bytes and the ~620ns/issue sync-engine
  serialization.  Wv, Wp are single fp16 DMAs.
- 6 contraction chunks (768 rows, no bias row).  Q/K biases are added by DVE
  during the PSUM->SBUF copies (per-partition tensor_scalar); the V and proj
  biases are folded into a host-side constant row (exact: softmax rows sum
  to 1) added during unsharding.
- exp() output and V are bf16 (E reaches ~75, fp16 would overflow);
  Q/K/x/W are fp16 (same 10-bit mantissa the old fp32r path kept).
- The gpsimd library and the ACT exp table are prewarmed at kernel start
  (v1 lazy-loaded both mid-kernel, costing a ~6us pipeline stall).
- QKV-phase PE work is interleaved INTO the attention phase: E(head0) runs
  right after the p3=0 Q/K blocks finish, so the ACT engine (the 55us exp
  bottleneck) starts ~35us earlier than v1's phase-sequential schedule.
- Softmax denominators come from a ones-column in each V slot.  AV PSUM is
  split into qc-halves so the next head's AV can start as soon as the first
  half is drained.  Head 5's reciprocal broadcast runs on the PE (ones
  matmul) so the output projection starts right behind it.
"""

import math
from contextlib import ExitStack

import numpy as np

import concourse.mybir as mybir
import concourse.tile as tile
from concourse import bacc
from concourse.bass_utils import run_bass_kernel_spmd

EMB = 768
HEADS = 12
DH = 64
N = 1024
B = 4
HPC = 6  # heads per core
CC = 6  # contraction chunks (768 = 6*128)
XO = 0  # xT cols in IN
QO = 1024  # Wq cols
KO = 1408  # Wk cols
BO = 1792  # bias cols (chunk 0 rows): q-bias head 0..5 | pad2 | k-bias head 0..5
INW = BO + 16
VW = HPC * 65 + 63  # V slots overlap: lhsT reads 128 cols from slot h*65
F32 = mybir.dt.float32
F32R = mybir.dt.float32r
F16 = mybir.dt.float16
BF16 = mybir.dt.bfloat16

N_CORES = 8
DEBUG = False


def build_program():
    nc = bacc.Bacc("TRN2", target_bir_lowering=False, debug=False, num_devices=N_CORES)

    IN = nc.dram_tensor("IN", [CC * 128, INW], F16, kind="ExternalInput").ap()
    Wv = nc.dram_tensor("Wv", [CC * 128, HPC * 65], F16, kind="ExternalInput").ap()
    Wp = nc.dram_tensor("Wp", [3 * 128, EMB], F16, kind="ExternalInput").ap()
    out = nc.dram_tensor("out", [N, EMB], F32, kind="ExternalOutput").ap()
    if DEBUG:
        dQT = nc.dram_tensor("dQT", [HPC * 128, N], F32, kind="ExternalOutput").ap()
        dKT = nc.dram_tensor("dKT", [HPC * 128, N], F32, kind="ExternalOutput").ap()
        dV = nc.dram_tensor("dV", [8 * 128, VW], F32, kind="ExternalOutput").ap()
        dOT = nc.dram_tensor("dOT", [3 * 128, N], F32, kind="ExternalOutput").ap()
        dEX = nc.dram_tensor("dEX", [128, N], F32, kind="ExternalOutput").ap()
        dOU = nc.dram_tensor("dOU", [128, N], F32, kind="ExternalOutput").ap()
        dPE = nc.dram_tensor("dPE", [128, N], F32, kind="ExternalOutput").ap()
        dRB = nc.dram_tensor("dRB", [128, N], F32, kind="ExternalOutput").ap()

    Exp = mybir.ActivationFunctionType.Exp

    with tile.TileContext(nc) as tc, ExitStack() as ctx:
        const = ctx.enter_context(tc.tile_pool(name="const", bufs=1))

        # ---- resident SBUF ------------------------------------------------
        IN_sb = const.tile([128, CC, INW], F16)
        Wv_sb = const.tile([128, CC, HPC * 65], F16)
        Wp_sb = const.tile([128, 3, EMB], F16)
        QT_sb = const.tile([128, HPC, N], F16)  # head h in parts 0..63, pad 64..127
        KT_sb = const.tile([128, HPC, N], F16)
        V_sb = const.tile([128, 8, VW], BF16)  # V natural, 8 token chunks
        OT_sb = const.tile([128, 3, N], F16)  # normalized O^T, head pair per chunk
        bias_sb = const.tile([128, 16], F32)
        warm = const.tile([128, 640], F16)  # junk matmul operand
        dummy = const.tile([128, 8], F32)  # prewarm sources
        dummy_a = const.tile([128, 8], F32)
        dummy_g = const.tile([128, 8], F32)
        ones64f = const.tile([1, 64], F32)
        ones64 = const.tile([1, 64], F32R)  # PE-broadcast stationary (head 5)
        if DEBUG:
            dbg_hold_ex = const.tile([128, N], F32)
            dbg_hold_pe = const.tile([128, N], F32)
            dbg_hold_ou = const.tile([128, N], F32)
            dbg_hold_rb = const.tile([128, N], F32)

        # prologue (engines otherwise idle during DMA bring-up).  DVE keeps
        # only what gates early work: warm (junk operand), the p3=0 head-slot
        # pads (E head 0/1 read them), V pad.  The p3=1,2 pads go to gpsimd,
        # whose queue is free until the head-0 broadcast (~35us in).
        nc.vector.memset(warm[:], 0.125)
        nc.vector.memset(dummy[:], 0.0)
        nc.vector.memset(ones64f[:], 1.0)
        nc.vector.tensor_copy(ones64[:], ones64f[:])
        nc.vector.memset(QT_sb[64:128, 0:2, :], 0.0)
        nc.vector.memset(KT_sb[64:128, 0:2, :], 0.0)
        nc.vector.memset(V_sb[:, :, HPC * 65 :], 0.0)
        nc.gpsimd.memset(QT_sb[64:128, 2:6, :], 0.0)
        nc.gpsimd.memset(KT_sb[64:128, 2:6, :], 0.0)
        # ACT exp-table prewarm (table DMA ~2.7us) + gpsimd library prewarm
        nc.scalar.activation(dummy_a[0:1, :], dummy[0:1, :], Exp)
        nc.gpsimd.partition_broadcast(dummy_g[0:64, :], dummy[0:1, :])

        # ---- input DMAs (6 sync + 2 scalar issues, in parallel) -----------
        IN_r = IN.rearrange("(c p) n -> p c n", p=128)
        for c in range(CC):
            nc.sync.dma_start(IN_sb[:, c, :], IN_r[:, c, :])
        nc.scalar.dma_start(Wv_sb[:], Wv.rearrange("(c p) n -> p c n", p=128))
        nc.scalar.dma_start(Wp_sb[:], Wp.rearrange("(c p) n -> p c n", p=128))
        # bias cast (waits on IN chunk 0)
        nc.vector.tensor_copy(bias_sb[:], IN_sb[:, 0, BO : BO + 16])

        # ---- PSUM pools (16KB/partition total) ----------------------------
        psO = ctx.enter_context(tc.tile_pool(name="psO", bufs=1, space="PSUM"))
        psA = tc.alloc_tile_pool(name="psA", bufs=1, space="PSUM")  # mm 2x2KB
        psB = tc.alloc_tile_pool(name="psB", bufs=1, space="PSUM")  # e 2x4KB

        expp = ctx.enter_context(tc.tile_pool(name="expp", bufs=10))
        oupool = ctx.enter_context(tc.tile_pool(name="oupool", bufs=3))
        rpool = ctx.enter_context(tc.tile_pool(name="rpool", bufs=2))

        # ---- emit helpers -------------------------------------------------
        def emit_junk(n, tag_tile):
            for _ in range(n):
                nc.tensor.matmul(
                    tag_tile[:, 0:512], warm[:, 0:128], warm[:, 128:640],
                    start=True, stop=True,
                )

        def qk_copy(g, qc, T_sb, p3, kbias):
            """PSUM group [128ch, 512 tok] -> two head slots (+ bias)."""
            bb = 8 if kbias else 0
            nc.vector.tensor_scalar_add(
                out=T_sb[0:64, 2 * p3, qc * 512 : (qc + 1) * 512],
                in0=g[0:64, :],
                scalar1=bias_sb[0:64, bb + 2 * p3 : bb + 2 * p3 + 1],
            )
            nc.vector.tensor_scalar_add(
                out=T_sb[0:64, 2 * p3 + 1, qc * 512 : (qc + 1) * 512],
                in0=g[64:128, :],
                scalar1=bias_sb[64:128, bb + 2 * p3 + 1 : bb + 2 * p3 + 2],
            )

        def emit_qk_group(p3, qc, wofs, T_sb, kbias, name):
            g = psA.tile([128, 512], F32, tag="mm", bufs=2, name=name)
            for c in range(CC):
                nc.tensor.matmul(
                    g[:],
                    IN_sb[:, c, wofs + p3 * 128 : wofs + (p3 + 1) * 128],
                    IN_sb[:, c, qc * 512 : (qc + 1) * 512],
                    start=(c == 0),
                    stop=(c == CC - 1),
                )
            qk_copy(g, qc, T_sb, p3, kbias)

        def emit_v_group(t8):
            g = psA.tile([128, 512], F32, tag="mm", bufs=2, name=f"psv_{t8}")
            for c in range(CC):
                nc.tensor.matmul(
                    g[:, 0 : HPC * 65],
                    IN_sb[:, c, t8 * 128 : (t8 + 1) * 128],
                    Wv_sb[:, c, :],
                    start=(c == 0),
                    stop=(c == CC - 1),
                )
            nc.vector.tensor_copy(V_sb[:, t8, 0 : HPC * 65], g[:, 0 : HPC * 65])
            # denominator ones-columns (the copy writes Wv's zero col there)
            nc.vector.memset(V_sb[:, t8, DH : HPC * 65 : 65], 1.0)

        def emit_e(h, kc):
            pe = psB.tile([128, N], F32, tag="e", bufs=2, name=f"pe_{h}_{kc}")
            for qc in range(2):
                nc.tensor.matmul(
                    pe[:, qc * 512 : (qc + 1) * 512],
                    KT_sb[:, h, kc * 128 : (kc + 1) * 128],
                    QT_sb[:, h, qc * 512 : (qc + 1) * 512],
                    start=True,
                    stop=True,
                )
            ex = expp.tile([128, N], BF16, tag="ex", name=f"ex_{h}_{kc}")
            nc.scalar.activation(ex[:], pe[:], Exp)
            if DEBUG and (h, kc) == (0, 0):
                nc.vector.tensor_copy(dbg_hold_ex[:], ex[:])
                nc.vector.tensor_copy(dbg_hold_pe[:], pe[:])
            return ex

        def begin_head(h):
            poa = psO.tile([65, 512], F32, tag="o2", bufs=2, name=f"poa_{h}")
            pob = psO.tile([65, 512], F32, tag="o2", bufs=2, name=f"pob_{h}")
            return (poa, pob)

        def emit_av(h, kc, po, ex):
            for qc in range(2):
                nc.tensor.matmul(
                    po[qc][0:65, :],
                    V_sb[:, kc, h * 65 : h * 65 + 65],
                    ex[:, qc * 512 : (qc + 1) * 512],
                    start=(kc == 0),
                    stop=(kc == 7),
                )

        def norm_chain(h, po, last=False):
            """po halves -> normalized OT chunk.  Ordered so the first PSUM
            half frees early (next head's AV reuses it) and rs lands early."""
            p3, half = divmod(h, 2)
            off = 64 * half
            ss = rpool.tile([1, N], F32, tag="ss", name=f"ss_{h}")
            ou = oupool.tile([65, N], F32, tag="ou", name=f"ou_{h}")
            rs = rpool.tile([1, N], F32, tag="rs", name=f"rs_{h}")
            if not last:
                nc.vector.tensor_copy(ss[:, 0:512], po[0][64:65, :])
                nc.vector.tensor_copy(ou[:, 0:512], po[0][0:65, :])
                nc.vector.tensor_copy(ou[:, 512:1024], po[1][0:65, :])
                nc.vector.tensor_copy(ss[:, 512:1024], po[1][64:65, :])
                if DEBUG and h == 0:
                    nc.vector.tensor_copy(dbg_hold_ou[0:65, :], ou[0:65, :])
                nc.vector.reciprocal_approx_fast(rs[:], ss[:])
                rb = rpool.tile([64, N], F32, tag="rb", name=f"rb_{h}")
                nc.gpsimd.partition_broadcast(rb[:], rs[:])
                nc.vector.tensor_mul(
                    OT_sb[off : off + 64, p3, :], ou[0:64, :], rb[:]
                )
                if DEBUG and h == 0:
                    nc.vector.tensor_copy(dbg_hold_rb[0:64, :], rb[:])
                return None
            # tail chain: sums first so rs (and the PE broadcast) land early
            nc.vector.tensor_copy(ss[:, 0:512], po[0][64:65, :])
            nc.vector.tensor_copy(ss[:, 512:1024], po[1][64:65, :])
            nc.vector.reciprocal_approx_fast(rs[:], ss[:])
            rsr = rpool.tile([1, N], F32R, tag="rsr", name="rsr")
            nc.vector.tensor_copy(rsr[:], rs[:])
            nc.vector.tensor_copy(ou[:, 0:512], po[0][0:65, :])
            nc.vector.tensor_copy(ou[:, 512:1024], po[1][0:65, :])
            return (ou, rsr, off, p3)

        # ---- phase 1a: junk + paced p3=0 Q groups -------------------------
        # Junk matmuls keep the PE's HAM activity window busy through the
        # DMA-paced stretch (an idle window declocks the PE to 1.2 GHz).
        jt = psB.tile([128, N], F32, tag="e", bufs=2, name="junk")
        emit_junk(12, jt)
        gQ0 = psA.tile([128, 512], F32, tag="mm", bufs=2, name="gQ0")
        gQ1 = psA.tile([128, 512], F32, tag="mm", bufs=2, name="gQ1")
        for c in range(CC):
            nc.tensor.matmul(
                gQ0[:], IN_sb[:, c, QO : QO + 128], IN_sb[:, c, 0:512],
                start=(c == 0), stop=(c == CC - 1),
            )
            nc.tensor.matmul(
                gQ1[:], IN_sb[:, c, QO : QO + 128], IN_sb[:, c, 512:1024],
                start=(c == 0), stop=(c == CC - 1),
            )
            if c < CC - 1:
                emit_junk(3, jt)
        qk_copy(gQ0, 0, QT_sb, 0, False)
        emit_junk(2, jt)
        qk_copy(gQ1, 1, QT_sb, 0, False)
        emit_qk_group(0, 0, KO, KT_sb, True, "gK0")
        emit_junk(2, jt)
        emit_qk_group(0, 1, KO, KT_sb, True, "gK1")

        # ---- phase 1b/2: interleave remaining QKV with head-0/1 attention -
        # QK p3=1,2 groups fill the E-head-0 stretch (their DMA data is in by
        # then); V groups fill the AV-head-0 loop one chunk ahead of use
        # (Wv lands later than IN, and AV(0,kc) needs V group kc written).
        fillers = [
            lambda: emit_qk_group(1, 0, QO, QT_sb, False, "gQ2"),
            lambda: emit_qk_group(1, 1, QO, QT_sb, False, "gQ3"),
            lambda: emit_qk_group(1, 0, KO, KT_sb, True, "gK2"),
            lambda: emit_qk_group(1, 1, KO, KT_sb, True, "gK3"),
            lambda: emit_qk_group(2, 0, QO, QT_sb, False, "gQ4"),
            lambda: emit_qk_group(2, 1, QO, QT_sb, False, "gQ5"),
            lambda: emit_qk_group(2, 0, KO, KT_sb, True, "gK4"),
            lambda: emit_qk_group(2, 1, KO, KT_sb, True, "gK5"),
        ] + [lambda t8=t8: emit_v_group(t8) for t8 in range(8)]
        fq = iter(fillers)

        def fill(n):
            for _ in range(n):
                f = next(fq, None)
                if f:
                    f()

        ex_tiles = {}
        ex_tiles[(0, 0)] = emit_e(0, 0)
        for kc in range(1, 8):
            fill(1)
            ex_tiles[(0, kc)] = emit_e(0, kc)
        fill(2)  # gK5 + V0

        # head 0 AVs interleaved with head-1 E and remaining QK groups
        po = begin_head(0)
        for kc in range(8):
            ex_tiles[(1, kc)] = emit_e(1, kc)
            fill(1)
            emit_av(0, kc, po, ex_tiles.pop((0, kc)))
        fill(99)  # leftovers, if any
        norm_chain(0, po)

        # heads 1..4: E two chunks ahead so the first AV of each head starts
        # after the previous head's PSUM halves have both drained
        for h in range(1, 5):
            po = begin_head(h)
            ex_tiles[(h + 1, 0)] = emit_e(h + 1, 0)
            ex_tiles[(h + 1, 1)] = emit_e(h + 1, 1)
            for kc in range(8):
                if kc + 2 < 8:
                    ex_tiles[(h + 1, kc + 2)] = emit_e(h + 1, kc + 2)
                emit_av(h, kc, po, ex_tiles.pop((h, kc)))
            norm_chain(h, po)

        # head 5: straight AVs (exps already pipelined ahead)
        po = begin_head(5)
        for kc in range(8):
            emit_av(5, kc, po, ex_tiles.pop((5, kc)))
        tail = norm_chain(5, po, last=True)

        # release attention PSUM; proj pool takes the freed 12KB
        psB.release()
        psA.release()
        psD = ctx.enter_context(tc.tile_pool(name="psD", bufs=1, space="PSUM"))
        outp = ctx.enter_context(tc.tile_pool(name="outp", bufs=3))

        # ---- phase 3: output projection -----------------------------------
        def proj_c(pso, qc8, cs):
            for c in cs:
                for n0, n1 in ((0, 512), (512, 768)):
                    nc.tensor.matmul(
                        pso[:, n0:n1],
                        OT_sb[:, c, qc8 * 128 : (qc8 + 1) * 128],
                        Wp_sb[:, c, n0:n1],
                        start=(c == 0),
                        stop=(c == 2),
                    )

        def proj_finish(pso, qc8):
            ot = outp.tile([128, EMB], F32, tag="out", name=f"ot_{qc8}")
            nc.vector.tensor_copy(ot[:], pso[:])
            nc.sync.dma_start(out[qc8 * 128 : (qc8 + 1) * 128, :], ot[:])

        # head-5 chain tail overlap: c0/c1 passes for the first blocks run
        # while ss/rs/broadcast/mul complete; the broadcast is a PE ones
        # matmul (waits only on rs) so OT chunk 2 lands mid-pass.
        ou5, rsr5, off5, p35 = tail
        pso_t = {}
        for blk in range(2):
            pso_t[blk] = psD.tile([128, EMB], F32, tag="pso", bufs=3, name=f"pso_{blk}")
            proj_c(pso_t[blk], blk, (0, 1))
        rb5 = [
            psO.tile([64, 512], F32, tag="o2", bufs=2, name=f"rb5_{qc}")
            for qc in range(2)
        ]
        for qc in range(2):
            nc.tensor.matmul(
                rb5[qc][:],
                ones64[:],
                rsr5[:, qc * 512 : (qc + 1) * 512],
                start=True,
                stop=True,
            )
        for qc in range(2):
            nc.vector.tensor_mul(
                OT_sb[off5 : off5 + 64, p35, qc * 512 : (qc + 1) * 512],
                ou5[0:64, qc * 512 : (qc + 1) * 512],
                rb5[qc][:],
            )
        pso_t[2] = psD.tile([128, EMB], F32, tag="pso", bufs=3, name="pso_2")
        proj_c(pso_t[2], 2, (0, 1))
        for blk in range(3):
            proj_c(pso_t[blk], blk, (2,))
            proj_finish(pso_t[blk], blk)
        for blk in range(3, 8):
            pso = psD.tile([128, EMB], F32, tag="pso", bufs=3, name=f"pso_{blk}")
            proj_c(pso, blk, (0, 1, 2))
            proj_finish(pso, blk)

        if DEBUG:
            dbg = ctx.enter_context(tc.tile_pool(name="dbg", bufs=2))
            def dump(dst, src_ap, nparts, width):
                t = dbg.tile([128, width], F32, tag="d", bufs=2)
                nc.vector.tensor_copy(t[0:nparts, :], src_ap)
                nc.sync.dma_start(dst, t[0:nparts, :])
            for hh in range(HPC):
                dump(dQT[hh * 128 : (hh + 1) * 128, :], QT_sb[:, hh, :], 128, N)
                dump(dKT[hh * 128 : (hh + 1) * 128, :], KT_sb[:, hh, :], 128, N)
            for t8 in range(8):
                dump(dV[t8 * 128 : (t8 + 1) * 128, :], V_sb[:, t8, :], 128, VW)
            for cc in range(3):
                dump(dOT[cc * 128 : (cc + 1) * 128, :], OT_sb[:, cc, :], 128, N)
            dump(dEX, dbg_hold_ex[:], 128, N)
            dump(dOU, dbg_hold_ou[:], 128, N)
            dump(dPE, dbg_hold_pe[:], 128, N)
            dump(dRB, dbg_hold_rb[:], 128, N)

    nc.compile()
    return nc


def build_in_maps(x, Wqkv, bqkv, Wproj, bproj):
    x = np.asarray(x, dtype=np.float32)
    Wqkv = np.asarray(Wqkv, dtype=np.float32)
    bqkv = np.asarray(bqkv, dtype=np.float32)
    Wproj = np.asarray(Wproj, dtype=np.float32)

    s = 1.0 / math.sqrt(EMB)
    cols = np.arange(3 * EMB).reshape(HEADS, DH, 3)  # (h, d, qkv) col index map
    in_maps = []
    for core in range(N_CORES):
        b, g = divmod(core, 2)
        hsl = slice(g * HPC, (g + 1) * HPC)
        qcols = cols[hsl, :, 0].reshape(-1)
        kcols = cols[hsl, :, 1].reshape(-1)
        vcols = cols[hsl, :, 2]  # [HPC, DH]

        IN_a = np.zeros((CC * 128, INW), np.float16)
        IN_a[:, XO : XO + N] = x[b].T
        IN_a[:, QO : QO + 384] = Wqkv[:, qcols]
        IN_a[:, KO : KO + 384] = Wqkv[:, kcols]
        # per-head bias columns (chunk-0 rows = partitions), duplicated in
        # both partition halves so either scalar-operand alignment works
        bq = bqkv[qcols].reshape(HPC, DH)
        bk = bqkv[kcols].reshape(HPC, DH)
        for j in range(HPC):
            IN_a[0:64, BO + j] = bq[j]
            IN_a[64:128, BO + j] = bq[j]
            IN_a[0:64, BO + 8 + j] = bk[j]
            IN_a[64:128, BO + 8 + j] = bk[j]

        Wv_a = np.zeros((CC * 128, HPC * 65), np.float16)
        for j in range(HPC):
            Wv_a[:, j * 65 : j * 65 + DH] = Wqkv[:, vcols[j]]

        Wp_a = (Wproj[g * 384 : (g + 1) * 384] * s).astype(np.float16)

        in_maps.append({"IN": IN_a, "Wv": Wv_a, "Wp": Wp_a})
    return in_maps


_NC_CACHE = None


def _get_program():
    global _NC_CACHE
    if _NC_CACHE is None:
        _NC_CACHE = build_program()
    return _NC_CACHE


def kernel(x, Wqkv, bqkv, Wproj, bproj, **_kwargs):
    nc = _get_program()
    in_maps = build_in_maps(x, Wqkv, bqkv, Wproj, bproj)
    res = run_bass_kernel_spmd(nc, in_maps, list(range(N_CORES))).results
    # V-bias and proj-bias fold (exact: normalized softmax rows sum to 1)
    bqkv64 = np.asarray(bqkv, dtype=np.float64)
    vcols = (np.arange(3 * EMB).reshape(HEADS, DH, 3))[:, :, 2].reshape(-1)
    vec = (
        bqkv64[vcols] @ np.asarray(Wproj, dtype=np.float64) / math.sqrt(EMB)
        + np.asarray(bproj, dtype=np.float64)
    ).astype(np.float32)
    out = np.empty((B, N, EMB), np.float32)
    for b in range(B):
        out[b] = res[2 * b]["out"] + res[2 * b + 1]["out"] + vec
    return out
